# revision 34
# baseline (speedup 1.0000x reference)
"""Multi-head self-attention (B=4, T=2048, D=1024, H=16) on 8 NeuronCores.

Sharding: batch x head-group. Core c handles batch b = c//2 and head group
g = c%2 (8 heads of 64 dims each, processed as 4 pairs of 2 row-tiled
heads). Host pre-transposes x and slices/transposes the weights; each core
computes its 8 heads' attention and a partial output projection; host sums
the two partials per batch and adds bo.

Single ACT-bound pipeline: exp volume (8 heads x 2048q x 2048k = 33.6M
elements/core, ScalarE-only at ~1.2ns/col for [128,1024] tiles) sets a
~315us floor, so all other engine work is arranged to hide under it.
  prologue: x -> SBUF (resident), v = x @ WvT_aug + bv (bf16, with a ones
            column per head so attn@V also accumulates the softmax sums),
            qT/kT for pair 0.
  main:     for each head pair j, per 512-q chunk, loop over 16 key tiles:
            row-tiled scores (K=64 pairs), ONE [128,1024] exp, ctx matmuls
            for the previous key tile (1-iter software pipeline so the
            in-order PE queue never blocks on the current exp). PE idle
            gaps are filled with real work popped from a queue: qk
            projection matmuls for pair j+1, and during j=3 the output
            projection of finished q-chunks.
  norm:     head-B ctx is written at partition offset 63 so its softmax-sum
            row (63) and head-A's (64) land in adjacent partitions; a K=2
            selector matmul broadcasts 1/S (reciprocal_approx_fast) to all
            128 partitions, one tensor_mul normalizes both heads at once.
  out-proj: bf16 (tolerance 2e-2 leaves plenty of headroom).

PSUM budget (8 banks): scores 2x[128,1024] (4) + ctx A/B [128,512] (2) +
filler q/k or out-proj accum (2); the broadcast matmul reuses an s slot.
"""

from collections import deque
from contextlib import ExitStack

import numpy as np
import ml_dtypes

import concourse.bass as bass
import concourse.mybir as mybir
import concourse.tile as tile
from concourse import bacc
from concourse.bass_utils import run_bass_kernel_spmd
from concourse.dve_ops import (RECIP_APPROX_FAST_CONSTS,
                               RECIPROCAL_APPROX_FAST)

F32 = mybir.dt.float32
F32R = mybir.dt.float32r
BF16 = mybir.dt.bfloat16
EXP = mybir.ActivationFunctionType.Exp
ADD = mybir.AluOpType.add
MULT = mybir.AluOpType.mult

B, T, D = 4, 2048, 1024
H, DH = 16, 64
G = 512            # head-group width (8 heads x 64)
GH = 8             # heads per group
P = 128
DK = D // P        # 8 contraction k-tiles for D
NKT = T // P       # 16 key tiles of 128
NQC = T // 512     # 4 q chunks of 512
VW = GH * (DH + 1)   # 520: v free width incl. ones columns
VC = VW // 2       # 260: v projection N-chunk (psum bank limit 512 fp32)


def r(ap):
    return ap.bitcast(F32R)


def emit_body(tc, io, parts='full'):
    nc = tc.nc
    xT, wq, wk, wv, wo = io["xT"], io["wq"], io["wk"], io["wv"], io["wo"]
    bq, bk, bv, out = io["bq"], io["bk"], io["bv"], io["out"]

    xT_r = xT.rearrange("(i p) t -> p i t", p=P)     # [128, 8, 2048]

    with ExitStack() as ectx:
        E = ectx.enter_context
        constp = E(tc.tile_pool(name="const", bufs=1))
        wp = E(tc.tile_pool(name="w", bufs=1))
        vp = E(tc.tile_pool(name="vsb", bufs=1))
        qkp = E(tc.tile_pool(name="qksb", bufs=1))
        cbfp = E(tc.tile_pool(name="cbf", bufs=1))
        ep = E(tc.tile_pool(name="e", bufs=4))
        cup = E(tc.tile_pool(name="cu", bufs=2))
        osp = E(tc.tile_pool(name="osb", bufs=2))

        # ---- constants / weights resident in SBUF ----
        bq_sb = constp.tile([P, 4], F32, name="bq_sb")
        nc.sync.dma_start(bq_sb[:], bq[:])
        bk_sb = constp.tile([P, 4], F32, name="bk_sb")
        nc.sync.dma_start(bk_sb[:], bk[:])
        bv_sb = constp.tile([P, VW], F32, name="bv_sb")
        nc.sync.dma_start(bv_sb[:], bv[:])
        # ones row at partition 64 for the K=1 1/S broadcast matmul
        ones_sb = constp.tile([P, 64], F32, name="ones_sb")
        nc.vector.memset(ones_sb[:], 1.0)
        # persistent rc tiles; row 64: cols 0-1023 = S_A|S_B copied from
        # psum (reciprocal_approx_fast can't read PSUM), 1024-2047 =
        # 1/S_A|1/S_B
        rc_t = [constp.tile([P, 2048], F32, name=f"rc_t{i}") for i in range(2)]
        nc.vector.memset(rc_t[0][64:65, :], 1.0)
        nc.vector.memset(rc_t[1][64:65, :], 1.0)

        # DMA order/queues: wv + x tg0 gate the v projection, so they go
        # first, split across both hardware queues (SP + ACT); wo is only
        # needed from j=3 on and goes last
        wv_sb = wp.tile([P, DK, VW], BF16, name="wv_sb")
        nc.sync.dma_start(wv_sb[:], wv.rearrange("(i p) m -> p i m", p=P))

        # exp table load happens at the first activation (~2.7us): pay it
        # here in the prologue instead of the first attention iteration
        e_warm = ep.tile([P, 1024], BF16, tag="e", name="e_warm")
        nc.scalar.activation(e_warm[:, 0:4], bq_sb[:], EXP)

        x_sb = wp.tile([P, DK, T], BF16, name="x_sb")
        wq_sb = wp.tile([P, DK, G], BF16, name="wq_sb")
        wk_sb = wp.tile([P, DK, G], BF16, name="wk_sb")
        wo_sb = wp.tile([P, 4, D], BF16, name="wo_sb")
        wq_r = wq.rearrange("(i p) m -> p i m", p=P)
        wk_r = wk.rearrange("(i p) m -> p i m", p=P)

        def dma_x(tg):
            for dk in range(DK):
                ts = slice(tg * 512, (tg + 1) * 512)
                eng = nc.sync if dk % 2 == 0 else nc.scalar
                eng.dma_start(x_sb[:, dk, ts], xT_r[:, dk, ts])

        def dma_wqk(j):
            js = slice(j * P, (j + 1) * P)
            nc.sync.dma_start(wq_sb[:, :, js], wq_r[:, :, js])
            nc.scalar.dma_start(wk_sb[:, :, js], wk_r[:, :, js])

        dma_x(0)
        dma_wqk(0)
        for tg in (1, 2, 3):
            dma_x(tg)
        for j in (1, 2, 3):
            dma_wqk(j)
        nc.scalar.dma_start(wo_sb[:], wo.rearrange("(i p) m -> p i m", p=P))

        v_sb = vp.tile([P, NKT, VW], BF16, name="v_sb")
        qT = [qkp.tile([P, T], BF16, name=f"qT{j}") for j in range(4)]
        kT = [qkp.tile([P, T], BF16, name=f"kT{j}") for j in range(4)]
        ctx_bf = [cbfp.tile([P, T], BF16, name=f"ctx{j}") for j in range(4)]

        # ---- prologue handled via filler closures below ----
        if parts == 'attn':
            nc.vector.memset(v_sb[:], 0.01)
            for j in range(4):
                nc.vector.memset(qT[j][:], 0.01)
                nc.vector.memset(kT[j][:], 0.01)

        # ---- main: ACT-bound attention with PE filler injection ----
        fillers = deque()       # (cost_ns, closure, seg)
        st_norm = {}
        budget = [0.0]

        def pump(slack_ns):
            budget[0] = min(budget[0] + slack_ns, 1500.0)
            while fillers and fillers[0][0] <= budget[0]:
                c, fn, _ = fillers.popleft()
                fn()
                budget[0] -= c

        def drain(n):
            for _ in range(n):
                if not fillers:
                    break
                fillers.popleft()[1]()

        with tc.tile_pool(name="pss", bufs=2, space="PSUM") as ps_s, \
             tc.tile_pool(name="psctx", bufs=2, space="PSUM") as ps_ctx, \
             tc.tile_pool(name="psfill", bufs=1, space="PSUM") as ps_fill:

            def make_v_fillers(tg):
                fs = []
                st = {}
                for ti in range(4):
                    for vc in range(2):
                        tag = "fq" if (ti * 2 + vc) % 2 == 0 else "fk"
                        for dk in range(DK):
                            def fv(tg=tg, ti=ti, vc=vc, dk=dk, tag=tag):
                                if dk == 0:
                                    st[ti, vc] = ps_fill.tile(
                                        [P, VC], F32, tag=tag,
                                        name=f"v{tg}_{ti}_{vc}")
                                xs = x_sb[:, dk, tg * 512 + ti * P:
                                          tg * 512 + (ti + 1) * P]
                                nc.tensor.matmul(
                                    st[ti, vc][:], xs,
                                    wv_sb[:, dk, vc * VC:(vc + 1) * VC],
                                    start=(dk == 0), stop=(dk == DK - 1))
                                if dk == DK - 1:
                                    nc.vector.tensor_add(
                                        v_sb[:, tg * 4 + ti,
                                             vc * VC:(vc + 1) * VC],
                                        st.pop((ti, vc))[:],
                                        bv_sb[:, vc * VC:(vc + 1) * VC])
                            fs.append((170.0, fv, 0))
                return fs

            def make_q_fillers(j, qcs, seg):
                fs = []
                st = {}
                for qc in qcs:
                    qs = slice(qc * 512, (qc + 1) * 512)
                    for dk in range(DK):
                        def fq(qc=qc, qs=qs, dk=dk):
                            if dk == 0:
                                st[qc] = ps_fill.tile(
                                    [P, 512], F32, tag="fq",
                                    name=f"pq{j}_{qc}")
                            nc.tensor.matmul(
                                st[qc][:], wq_sb[:, dk, j * P:(j + 1) * P],
                                x_sb[:, dk, qs],
                                start=(dk == 0), stop=(dk == DK - 1))
                            if dk == DK - 1:
                                nc.vector.tensor_scalar(
                                    qT[j][:, qs], st.pop(qc)[:],
                                    bq_sb[:, j:j + 1], 0.125, ADD, MULT)
                        fs.append((270.0, fq, seg))
                return fs

            def make_k_fillers(j, qcs, seg):
                fs = []
                st = {}
                for qc in qcs:
                    qs = slice(qc * 512, (qc + 1) * 512)
                    for dk in range(DK):
                        def fk(qc=qc, qs=qs, dk=dk):
                            if dk == 0:
                                st[qc] = ps_fill.tile(
                                    [P, 512], F32, tag="fk",
                                    name=f"pk{j}_{qc}")
                            nc.tensor.matmul(
                                st[qc][:], wk_sb[:, dk, j * P:(j + 1) * P],
                                x_sb[:, dk, qs],
                                start=(dk == 0), stop=(dk == DK - 1))
                            if dk == DK - 1:
                                nc.vector.tensor_scalar_add(
                                    kT[j][:, qs], st.pop(qc)[:],
                                    bk_sb[:, j:j + 1])
                        fs.append((270.0, fk, seg))
                return fs

            def make_qk_fillers(j):
                fs = []
                st = {}
                for qc in range(NQC):
                    qs = slice(qc * 512, (qc + 1) * 512)
                    for dk in range(DK):
                        def fq(qc=qc, qs=qs, dk=dk):
                            if dk == 0:
                                st["q", qc] = ps_fill.tile(
                                    [P, 512], F32, tag="fq", name=f"fq{j}_{qc}")
                            nc.tensor.matmul(
                                st["q", qc][:], wq_sb[:, dk, j * P:(j + 1) * P],
                                x_sb[:, dk, qs],
                                start=(dk == 0), stop=(dk == DK - 1))
                            if dk == DK - 1:
                                nc.vector.tensor_scalar(
                                    qT[j][:, qs], st["q", qc][:],
                                    bq_sb[:, j:j + 1], 0.125, ADD, MULT)

                        def fk(qc=qc, qs=qs, dk=dk):
                            if dk == 0:
                                st["k", qc] = ps_fill.tile(
                                    [P, 512], F32, tag="fk", name=f"fk{j}_{qc}")
                            nc.tensor.matmul(
                                st["k", qc][:], wk_sb[:, dk, j * P:(j + 1) * P],
                                x_sb[:, dk, qs],
                                start=(dk == 0), stop=(dk == DK - 1))
                            if dk == DK - 1:
                                nc.vector.tensor_scalar_add(
                                    kT[j][:, qs], st["k", qc][:],
                                    bk_sb[:, j:j + 1])
                        fs.append((270.0, fk, j))
                        fs.append((270.0, fq, j))
                return fs

            def make_oproj_fillers(qc):
                fs = []
                st = {}
                for qt4 in range(4):
                    qtile = qc * 4 + qt4
                    tsl = slice(qtile * P, (qtile + 1) * P)
                    for dc in range(2):
                        tag = "fq" if (qt4 * 2 + dc) % 2 == 0 else "fk"
                        dsl = slice(dc * 512, (dc + 1) * 512)

                        def f1(qtile=qtile, tsl=tsl, dc=dc, tag=tag, dsl=dsl):
                            o_ps = ps_fill.tile([P, 512], F32, tag=tag,
                                                name=f"o{qtile}_{dc}")
                            st[qtile, dc] = o_ps
                            for j in (0, 1):
                                nc.tensor.matmul(
                                    o_ps[:], ctx_bf[j][:, tsl],
                                    wo_sb[:, j, dsl],
                                    start=(j == 0), stop=False)

                        def f2(qtile=qtile, tsl=tsl, dc=dc, dsl=dsl):
                            o_ps = st[qtile, dc]
                            for j in (2, 3):
                                nc.tensor.matmul(
                                    o_ps[:], ctx_bf[j][:, tsl],
                                    wo_sb[:, j, dsl],
                                    start=False, stop=(j == 3))
                            o_sb = osp.tile([P, 512], BF16, tag="ob",
                                            name=f"ob{qtile}_{dc}")
                            nc.vector.tensor_copy(o_sb[:], o_ps[:])
                            eng = nc.sync if dc == 0 else nc.scalar
                            eng.dma_start(out[tsl, dsl], o_sb[:])
                        fs.append((540.0, f1, 99))
                        fs.append((540.0, f2, 99))
                return fs

            # mini-prologue: v tile-group 0 + q/k for (pair 0, chunk 0)
            # emitted serially; the rest of v/qk(pair 0) queues as fillers
            # drained on a fixed per-iteration quota during (j=0, qc=0) so
            # the exp stream starts ~16us in and overlaps the prologue.
            # k chunks go first (scores(kt) needs kT ahead of ctx's v).
            if parts != 'attn':
                for fx in (make_v_fillers(0) + make_q_fillers(0, [0], 0)
                           + make_k_fillers(0, [0], 0)):
                    fx[1]()
                fillers.extend(make_k_fillers(0, [1, 2, 3], 0))
                for tg in (1, 2, 3):
                    fillers.extend(make_v_fillers(tg))
                fillers.extend(make_q_fillers(0, [1, 2, 3], 0))
            if parts == 'pro':
                while fillers:
                    fillers.popleft()[1]()
                return

            for j in range(4):
                if j > 0:
                    # anything attention(j) depends on must be emitted now
                    while fillers and fillers[0][2] <= j:
                        fillers.popleft()[1]()
                if j < 3 and parts == 'full':
                    fillers.extend(make_qk_fillers(j + 1))
                for qc in range(NQC):
                    qs = slice(qc * 512, (qc + 1) * 512)
                    ctxA = ps_ctx.tile([P, 512], F32, tag="ctx",
                                       name=f"cA{j}_{qc}")
                    ctxB = ps_ctx.tile([P, 512], F32, tag="ctx",
                                       name=f"cB{j}_{qc}")

                    def ctx_mms(e, kt, j=j, ctxA=ctxA, ctxB=ctxB):
                        nc.tensor.matmul(
                            ctxA[0:65, :], v_sb[:, kt, j * 130:j * 130 + 65],
                            e[:, 0:512],
                            start=(kt == 0), stop=(kt == NKT - 1))
                        nc.tensor.matmul(
                            ctxB[0:65, :],
                            v_sb[:, kt, j * 130 + 65:j * 130 + 130],
                            e[:, 512:1024],
                            start=(kt == 0), stop=(kt == NKT - 1))

                    # 2-iter software pipeline: ctx consumes e from two
                    # key-tiles back so the in-order PE queue never parks on
                    # the current exp; fillers slot into the exp-wait window
                    pipe = []
                    for kt in range(NKT):
                        ks = slice(kt * P, (kt + 1) * P)
                        s = ps_s.tile([P, 1024], F32, tag="s",
                                      name=f"s{j}_{qc}_{kt}")
                        nc.tensor.matmul(s[:, 0:512], kT[j][0:64, ks],
                                         qT[j][0:64, qs], start=True,
                                         stop=True, tile_position=(0, 0))
                        nc.tensor.matmul(s[:, 512:1024], kT[j][64:128, ks],
                                         qT[j][64:128, qs], start=True,
                                         stop=True, tile_position=(64, 0))
                        e = ep.tile([P, 1024], BF16, tag="e",
                                    name=f"e{j}_{qc}_{kt}")
                        nc.scalar.activation(e[:], s[:], EXP)
                        if j == 0 and qc == 0:
                            drain(20)
                        else:
                            pump(680.0)
                        if len(pipe) >= 2:
                            ctx_mms(*pipe.pop(0))
                        pipe.append((e, kt))
                    for pr in pipe:
                        ctx_mms(*pr)

                    # normalization: both softmax sums sit at psum row 64.
                    # Immediate part (frees the ctx psum slots): copy the S
                    # rows and the raw ctx into SBUF. Deferred part (pushed
                    # to the front of the filler queue so it runs inside the
                    # next chunk's iterations while ACT stays busy): approx
                    # reciprocal of both sums (full-partition op --
                    # reciprocal_approx_fast no-ops on single-row slices;
                    # rows != 64 hold 1.0 and are discarded), one K=1 matmul
                    # per head broadcasts 1/S to partitions 0-63, two muls
                    # normalize, and head B's normalized tile moves to
                    # partitions 64-127 via sbuf-to-sbuf DMA (engines can't
                    # shift partitions; DMA is idle here).
                    rc = rc_t[(j * NQC + qc) % 2]
                    nc.vector.tensor_copy(rc[64:65, 0:512], ctxA[64:65, :])
                    nc.vector.tensor_copy(rc[64:65, 512:1024], ctxB[64:65, :])
                    cu = cup.tile([P, 512], F32, tag="cu", name=f"cu{j}_{qc}")
                    nc.vector.tensor_copy(cu[0:64, :], ctxA[0:64, :])
                    cuB = cup.tile([P, 512], F32, tag="cuB", name=f"cB{j}_{qc}")
                    nc.vector.tensor_copy(cuB[0:64, :], ctxB[0:64, :])

                    def d1(j=j, qc=qc, rc=rc):
                        nc.vector.reciprocal_approx_fast(rc[:, 1024:2048],
                                                         rc[:, 0:1024])
                        bc = ps_s.tile([P, 1024], F32, tag="s",
                                       name=f"bc{j}_{qc}")
                        nc.tensor.matmul(bc[0:64, 0:512], ones_sb[64:65, :],
                                         rc[64:65, 1024:1536],
                                         start=True, stop=True)
                        nc.tensor.matmul(bc[0:64, 512:1024], ones_sb[64:65, :],
                                         rc[64:65, 1536:2048],
                                         start=True, stop=True)
                        st_norm[j, qc] = bc

                    def d2(j=j, qc=qc, qs=qs, cu=cu, cuB=cuB):
                        bc = st_norm.pop((j, qc))
                        nc.vector.tensor_mul(ctx_bf[j][0:64, qs], cu[0:64, :],
                                             bc[0:64, 0:512])
                        tmpB = cup.tile([P, 512], BF16, tag="tB",
                                        name=f"tB{j}_{qc}")
                        nc.vector.tensor_mul(tmpB[0:64, :], cuB[0:64, :],
                                             bc[0:64, 512:1024])
                        nc.sync.dma_start(ctx_bf[j][64:128, qs], tmpB[0:64, :])
                    fillers.extendleft([(150.0, d2, j), (760.0, d1, j)])
                    if j == 3 and parts == 'full':
                        fillers.extend(make_oproj_fillers(qc))

            # tail: drain remaining fillers (out-proj of the last q chunk)
            while fillers:
                fillers.popleft()[1]()


def build(loop_k: int = 1, parts: str = 'full'):
    nc = bacc.Bacc("TRN2", target_bir_lowering=False, debug=False)
    io = {
        "xT": nc.dram_tensor("xT", [D, T], BF16, kind="ExternalInput").ap(),
        "wq": nc.dram_tensor("wq", [D, G], BF16, kind="ExternalInput").ap(),
        "wk": nc.dram_tensor("wk", [D, G], BF16, kind="ExternalInput").ap(),
        "wv": nc.dram_tensor("wv", [D, VW], BF16, kind="ExternalInput").ap(),
        "wo": nc.dram_tensor("wo", [G, D], BF16, kind="ExternalInput").ap(),
        "bq": nc.dram_tensor("bq", [P, 4], F32, kind="ExternalInput").ap(),
        "bk": nc.dram_tensor("bk", [P, 4], F32, kind="ExternalInput").ap(),
        "bv": nc.dram_tensor("bv", [P, VW], F32, kind="ExternalInput").ap(),
        "out": nc.dram_tensor("out", [T, D], BF16, kind="ExternalOutput").ap(),
    }
    with tile.TileContext(nc) as tc:
        if loop_k == 1:
            emit_body(tc, io, parts)
        else:
            with tc.For_i(0, loop_k, 1):
                emit_body(tc, io, parts)
    nc.compile()
    return nc


def prep_inputs(x, Wq, bq, Wk, bk, Wv, bv, Wo, bo):
    """Host-side sharding: returns in_maps for cores 0..7."""
    f = np.float32
    bf = ml_dtypes.bfloat16
    in_maps = []
    for c in range(8):
        b, g = c // 2, c % 2
        gs = slice(g * G, (g + 1) * G)
        wv_aug = np.zeros((D, VW), f)
        bv_aug = np.zeros((VW,), f)
        wv_g = np.ascontiguousarray(Wv[gs, :].T)        # [D, 512]
        for h in range(GH):      # per head: [v(64), ones] -> S at row 64
            base = h * 65
            wv_aug[:, base:base + 64] = wv_g[:, h * 64:(h + 1) * 64]
            bv_aug[base:base + 64] = bv[gs][h * 64:(h + 1) * 64]
            bv_aug[base + 64] = 1.0
        in_maps.append({
            "xT": np.ascontiguousarray(np.asarray(x[b]).T).astype(bf),
            "wq": np.ascontiguousarray(Wq[gs, :].T).astype(bf),
            "wk": np.ascontiguousarray(Wk[gs, :].T).astype(bf),
            "wv": wv_aug.astype(bf),
            "wo": np.ascontiguousarray(Wo[:, gs].T).astype(bf),
            "bq": np.ascontiguousarray(bq[gs].reshape(4, P).T),
            "bk": np.ascontiguousarray(bk[gs].reshape(4, P).T),
            "bv": np.broadcast_to(bv_aug, (P, VW)).copy(),
        })
    return in_maps


def gather_output(results, bo):
    out = np.empty((B, T, D), np.float32)
    for b in range(B):
        out[b] = (results[2 * b]["out"].astype(np.float32)
                  + results[2 * b + 1]["out"].astype(np.float32)
                  + np.asarray(bo)[None, :])
    return out


_nc_cache = {}


def kernel(x, Wq, bq, Wk, bk, Wv, bv, Wo, bo):
    if "nc" not in _nc_cache:
        _nc_cache["nc"] = build()
    nc = _nc_cache["nc"]
    in_maps = prep_inputs(x, Wq, bq, Wk, bk, Wv, bv, Wo, bo)
    res = run_bass_kernel_spmd(nc, in_maps, list(range(8)))
    return gather_output(res.results, bo)


# revision 35
# speedup vs baseline: 1.1887x; 1.1887x over previous
"""Multi-head self-attention (B=4, T=2048, D=1024, H=16) on 8 NeuronCores.

Sharding: batch x head-group. Core c handles batch b = c//2 and head group
g = c%2 (8 heads of 64 dims each, processed as 4 pairs of 2 row-tiled
heads). Host pre-transposes x and slices/transposes the weights; each core
computes its 8 heads' attention and a partial output projection; host sums
the two partials per batch and adds bo.

Single ACT-bound pipeline: exp volume (8 heads x 2048q x 2048k = 33.6M
elements/core, ScalarE-only at ~1.2ns/col for [128,1024] tiles) sets a
~315us floor, so all other engine work is arranged to hide under it.
  prologue: x -> SBUF (resident), v = x @ WvT_aug + bv (bf16, with a ones
            column per head so attn@V also accumulates the softmax sums),
            qT/kT for pair 0.
  main:     for each head pair j, per 512-q chunk, loop over 16 key tiles:
            row-tiled scores (K=64 pairs), ONE [128,1024] exp, ctx matmuls
            for the previous key tile (1-iter software pipeline so the
            in-order PE queue never blocks on the current exp). PE idle
            gaps are filled with real work popped from a queue: qk
            projection matmuls for pair j+1, and during j=3 the output
            projection of finished q-chunks.
  norm:     head-B ctx is written at partition offset 63 so its softmax-sum
            row (63) and head-A's (64) land in adjacent partitions; a K=2
            selector matmul broadcasts 1/S (reciprocal_approx_fast) to all
            128 partitions, one tensor_mul normalizes both heads at once.
  out-proj: bf16 (tolerance 2e-2 leaves plenty of headroom).

PSUM budget (8 banks): scores 2x[128,1024] (4) + ctx A/B [128,512] (2) +
filler q/k or out-proj accum (2); the broadcast matmul reuses an s slot.
"""

from collections import deque
from contextlib import ExitStack

import numpy as np
import ml_dtypes

import concourse.bass as bass
import concourse.mybir as mybir
import concourse.tile as tile
from concourse import bacc
from concourse.bass_utils import run_bass_kernel_spmd
from concourse.dve_ops import (RECIP_APPROX_FAST_CONSTS,
                               RECIPROCAL_APPROX_FAST)

F32 = mybir.dt.float32
F32R = mybir.dt.float32r
BF16 = mybir.dt.bfloat16
EXP = mybir.ActivationFunctionType.Exp
ADD = mybir.AluOpType.add
MULT = mybir.AluOpType.mult

B, T, D = 4, 2048, 1024
H, DH = 16, 64
G = 512            # head-group width (8 heads x 64)
GH = 8             # heads per group
P = 128
DK = D // P        # 8 contraction k-tiles for D
NKT = T // P       # 16 key tiles of 128
NQC = T // 512     # 4 q chunks of 512
VW = GH * (DH + 1)   # 520: v free width incl. ones columns
VC = VW // 2       # 260: v projection N-chunk (psum bank limit 512 fp32)


def r(ap):
    return ap.bitcast(F32R)


def emit_body(tc, io, parts='full'):
    nc = tc.nc
    xT, wq, wk, wv, wo = io["xT"], io["wq"], io["wk"], io["wv"], io["wo"]
    bq, bk, bv, out = io["bq"], io["bk"], io["bv"], io["out"]

    xT_r = xT.rearrange("(i p) t -> p i t", p=P)     # [128, 8, 2048]

    with ExitStack() as ectx:
        E = ectx.enter_context
        constp = E(tc.tile_pool(name="const", bufs=1))
        wp = E(tc.tile_pool(name="w", bufs=1))
        vp = E(tc.tile_pool(name="vsb", bufs=1))
        qkp = E(tc.tile_pool(name="qksb", bufs=1))
        cbfp = E(tc.tile_pool(name="cbf", bufs=1))
        ep = E(tc.tile_pool(name="e", bufs=4))
        cup = E(tc.tile_pool(name="cu", bufs=2))
        osp = E(tc.tile_pool(name="osb", bufs=2))

        # ---- constants / weights resident in SBUF ----
        bq_sb = constp.tile([P, 4], F32, name="bq_sb")
        nc.sync.dma_start(bq_sb[:], bq[:])
        bk_sb = constp.tile([P, 4], F32, name="bk_sb")
        nc.sync.dma_start(bk_sb[:], bk[:])
        bv_sb = constp.tile([P, VW], F32, name="bv_sb")
        nc.sync.dma_start(bv_sb[:], bv[:])
        # ones row at partition 64 for the K=1 1/S broadcast matmul
        ones_sb = constp.tile([P, 64], F32, name="ones_sb")
        nc.vector.memset(ones_sb[:], 1.0)
        # persistent rc tiles; row 64: cols 0-1023 = S_A|S_B copied from
        # psum (reciprocal_approx_fast can't read PSUM), 1024-2047 =
        # 1/S_A|1/S_B
        rc_t = [constp.tile([P, 2048], F32, name=f"rc_t{i}") for i in range(2)]
        nc.vector.memset(rc_t[0][64:65, :], 1.0)
        nc.vector.memset(rc_t[1][64:65, :], 1.0)

        # DMA order/queues: wv + x tg0 gate the v projection, so they go
        # first, split across both hardware queues (SP + ACT); wo is only
        # needed from j=3 on and goes last
        wv_sb = wp.tile([P, DK, VW], BF16, name="wv_sb")
        nc.sync.dma_start(wv_sb[:], wv.rearrange("(i p) m -> p i m", p=P))

        # exp table load happens at the first activation (~2.7us): pay it
        # here in the prologue instead of the first attention iteration
        e_warm = ep.tile([P, 1024], BF16, tag="e", name="e_warm")
        nc.scalar.activation(e_warm[:, 0:4], bq_sb[:], EXP)

        x_sb = wp.tile([P, DK, T], BF16, name="x_sb")
        wq_sb = wp.tile([P, DK, G], BF16, name="wq_sb")
        wk_sb = wp.tile([P, DK, G], BF16, name="wk_sb")
        wo_sb = wp.tile([P, 4, D], BF16, name="wo_sb")
        wq_r = wq.rearrange("(i p) m -> p i m", p=P)
        wk_r = wk.rearrange("(i p) m -> p i m", p=P)

        def dma_x(tg):
            for dk in range(DK):
                ts = slice(tg * 512, (tg + 1) * 512)
                eng = nc.sync if dk % 2 == 0 else nc.scalar
                eng.dma_start(x_sb[:, dk, ts], xT_r[:, dk, ts])

        def dma_wqk(j):
            js = slice(j * P, (j + 1) * P)
            nc.sync.dma_start(wq_sb[:, :, js], wq_r[:, :, js])
            nc.scalar.dma_start(wk_sb[:, :, js], wk_r[:, :, js])

        dma_x(0)
        dma_wqk(0)
        for tg in (1, 2, 3):
            dma_x(tg)
        for j in (1, 2, 3):
            dma_wqk(j)
        nc.scalar.dma_start(wo_sb[:], wo.rearrange("(i p) m -> p i m", p=P))

        v_sb = vp.tile([P, NKT, VW], BF16, name="v_sb")
        qT = [qkp.tile([P, T], BF16, name=f"qT{j}") for j in range(4)]
        kT = [qkp.tile([P, T], BF16, name=f"kT{j}") for j in range(4)]
        ctx_bf = [cbfp.tile([P, T], BF16, name=f"ctx{j}") for j in range(4)]

        # ---- prologue: v projection (all 8 heads), [t, hd] layout ----
        if parts == 'attn':
            nc.vector.memset(v_sb[:], 0.01)
            for j in range(4):
                nc.vector.memset(qT[j][:], 0.01)
                nc.vector.memset(kT[j][:], 0.01)
        if parts != 'attn':
         with tc.tile_pool(name="psv", bufs=8, space="PSUM") as ps_v:
            for tg in range(4):
                v_ps = [[ps_v.tile([P, VC], F32, tag="vps",
                                   name=f"vps_{tg}_{ti}_{vc}")
                         for vc in range(2)] for ti in range(4)]
                for dk in range(DK):
                    for ti in range(4):
                        xs = x_sb[:, dk, tg * 512 + ti * P:tg * 512 + (ti + 1) * P]
                        for vc in range(2):
                            nc.tensor.matmul(
                                v_ps[ti][vc][:], xs,
                                wv_sb[:, dk, vc * VC:(vc + 1) * VC],
                                start=(dk == 0), stop=(dk == DK - 1))
                for ti in range(4):
                    for vc in range(2):
                        nc.vector.tensor_add(
                            v_sb[:, tg * 4 + ti, vc * VC:(vc + 1) * VC],
                            v_ps[ti][vc][:], bv_sb[:, vc * VC:(vc + 1) * VC])

        # ---- prologue: qT/kT for pair 0 ----
        if parts != 'attn':
         with tc.tile_pool(name="psqk0", bufs=2, space="PSUM") as ps_qk0:
            for qc in range(NQC):
                qs = slice(qc * 512, (qc + 1) * 512)
                q_ps = ps_qk0.tile([P, 512], F32, tag="q0", bufs=1)
                k_ps = ps_qk0.tile([P, 512], F32, tag="k0", bufs=1)
                for dk in range(DK):
                    nc.tensor.matmul(q_ps[:], wq_sb[:, dk, 0:P],
                                     x_sb[:, dk, qs],
                                     start=(dk == 0), stop=(dk == DK - 1))
                    nc.tensor.matmul(k_ps[:], wk_sb[:, dk, 0:P],
                                     x_sb[:, dk, qs],
                                     start=(dk == 0), stop=(dk == DK - 1))
                # scores scale 1/sqrt(dh)=1/8 folded into q so exp runs raw
                nc.vector.tensor_scalar(qT[0][:, qs], q_ps[:],
                                        bq_sb[:, 0:1], 0.125, ADD, MULT)
                nc.vector.tensor_scalar_add(kT[0][:, qs], k_ps[:],
                                            bk_sb[:, 0:1])

        # ---- main: ACT-bound attention with PE filler injection ----
        fillers = deque()       # (cost_ns, closure, seg)
        st_norm = {}
        budget = [0.0]

        def pump(slack_ns):
            budget[0] = min(budget[0] + slack_ns, 1500.0)
            while fillers and fillers[0][0] <= budget[0]:
                c, fn, _ = fillers.popleft()
                fn()
                budget[0] -= c

        with tc.tile_pool(name="pss", bufs=2, space="PSUM") as ps_s, \
             tc.tile_pool(name="psctx", bufs=2, space="PSUM") as ps_ctx, \
             tc.tile_pool(name="psfill", bufs=1, space="PSUM") as ps_fill:

            def make_qk_fillers(j):
                fs = []
                st = {}
                for qc in range(NQC):
                    qs = slice(qc * 512, (qc + 1) * 512)
                    for dk in range(DK):
                        def fq(qc=qc, qs=qs, dk=dk):
                            if dk == 0:
                                st["q", qc] = ps_fill.tile(
                                    [P, 512], F32, tag="fq", name=f"fq{j}_{qc}")
                            nc.tensor.matmul(
                                st["q", qc][:], wq_sb[:, dk, j * P:(j + 1) * P],
                                x_sb[:, dk, qs],
                                start=(dk == 0), stop=(dk == DK - 1))
                            if dk == DK - 1:
                                nc.vector.tensor_scalar(
                                    qT[j][:, qs], st["q", qc][:],
                                    bq_sb[:, j:j + 1], 0.125, ADD, MULT)

                        def fk(qc=qc, qs=qs, dk=dk):
                            if dk == 0:
                                st["k", qc] = ps_fill.tile(
                                    [P, 512], F32, tag="fk", name=f"fk{j}_{qc}")
                            nc.tensor.matmul(
                                st["k", qc][:], wk_sb[:, dk, j * P:(j + 1) * P],
                                x_sb[:, dk, qs],
                                start=(dk == 0), stop=(dk == DK - 1))
                            if dk == DK - 1:
                                nc.vector.tensor_scalar_add(
                                    kT[j][:, qs], st["k", qc][:],
                                    bk_sb[:, j:j + 1])
                        fs.append((270.0, fk, j))
                        fs.append((270.0, fq, j))
                return fs

            def make_oproj_fillers(qc):
                fs = []
                st = {}
                for qt4 in range(4):
                    qtile = qc * 4 + qt4
                    tsl = slice(qtile * P, (qtile + 1) * P)
                    for dc in range(2):
                        tag = "fq" if (qt4 * 2 + dc) % 2 == 0 else "fk"
                        dsl = slice(dc * 512, (dc + 1) * 512)

                        def f1(qtile=qtile, tsl=tsl, dc=dc, tag=tag, dsl=dsl):
                            o_ps = ps_fill.tile([P, 512], F32, tag=tag,
                                                name=f"o{qtile}_{dc}")
                            st[qtile, dc] = o_ps
                            for j in (0, 1):
                                nc.tensor.matmul(
                                    o_ps[:], ctx_bf[j][:, tsl],
                                    wo_sb[:, j, dsl],
                                    start=(j == 0), stop=False)

                        def f2(qtile=qtile, tsl=tsl, dc=dc, dsl=dsl):
                            o_ps = st[qtile, dc]
                            for j in (2, 3):
                                nc.tensor.matmul(
                                    o_ps[:], ctx_bf[j][:, tsl],
                                    wo_sb[:, j, dsl],
                                    start=False, stop=(j == 3))
                            o_sb = osp.tile([P, 512], BF16, tag="ob",
                                            name=f"ob{qtile}_{dc}")
                            nc.vector.tensor_copy(o_sb[:], o_ps[:])
                            eng = nc.sync if dc == 0 else nc.scalar
                            eng.dma_start(out[tsl, dsl], o_sb[:])
                        fs.append((540.0, f1, 99))
                        fs.append((540.0, f2, 99))
                return fs

            for j in range(4):
                if parts == 'pro':
                    break
                if j < 3 and parts == 'full':
                    fillers.extend(make_qk_fillers(j + 1))
                for qc in range(NQC):
                    qs = slice(qc * 512, (qc + 1) * 512)
                    ctxA = ps_ctx.tile([P, 512], F32, tag="ctx",
                                       name=f"cA{j}_{qc}")
                    ctxB = ps_ctx.tile([P, 512], F32, tag="ctx",
                                       name=f"cB{j}_{qc}")

                    def ctx_mms(e, kt, j=j, ctxA=ctxA, ctxB=ctxB):
                        nc.tensor.matmul(
                            ctxA[0:65, :], v_sb[:, kt, j * 130:j * 130 + 65],
                            e[:, 0:512],
                            start=(kt == 0), stop=(kt == NKT - 1))
                        nc.tensor.matmul(
                            ctxB[0:65, :],
                            v_sb[:, kt, j * 130 + 65:j * 130 + 130],
                            e[:, 512:1024],
                            start=(kt == 0), stop=(kt == NKT - 1))

                    # 2-iter software pipeline: ctx consumes e from two
                    # key-tiles back so the in-order PE queue never parks on
                    # the current exp; fillers slot into the exp-wait window
                    pipe = []
                    for kt in range(NKT):
                        ks = slice(kt * P, (kt + 1) * P)
                        s = ps_s.tile([P, 1024], F32, tag="s",
                                      name=f"s{j}_{qc}_{kt}")
                        nc.tensor.matmul(s[:, 0:512], kT[j][0:64, ks],
                                         qT[j][0:64, qs], start=True,
                                         stop=True, tile_position=(0, 0))
                        nc.tensor.matmul(s[:, 512:1024], kT[j][64:128, ks],
                                         qT[j][64:128, qs], start=True,
                                         stop=True, tile_position=(64, 0))
                        e = ep.tile([P, 1024], BF16, tag="e",
                                    name=f"e{j}_{qc}_{kt}")
                        nc.scalar.activation(e[:], s[:], EXP)
                        pump(680.0)
                        if len(pipe) >= 2:
                            ctx_mms(*pipe.pop(0))
                        pipe.append((e, kt))
                    for pr in pipe:
                        ctx_mms(*pr)

                    # normalization: both softmax sums sit at psum row 64.
                    # Immediate part (frees the ctx psum slots): copy the S
                    # rows and the raw ctx into SBUF. Deferred part (pushed
                    # to the front of the filler queue so it runs inside the
                    # next chunk's iterations while ACT stays busy): approx
                    # reciprocal of both sums (full-partition op --
                    # reciprocal_approx_fast no-ops on single-row slices;
                    # rows != 64 hold 1.0 and are discarded), one K=1 matmul
                    # per head broadcasts 1/S to partitions 0-63, two muls
                    # normalize, and head B's normalized tile moves to
                    # partitions 64-127 via sbuf-to-sbuf DMA (engines can't
                    # shift partitions; DMA is idle here).
                    rc = rc_t[(j * NQC + qc) % 2]
                    nc.vector.tensor_copy(rc[64:65, 0:512], ctxA[64:65, :])
                    nc.vector.tensor_copy(rc[64:65, 512:1024], ctxB[64:65, :])
                    cu = cup.tile([P, 512], F32, tag="cu", name=f"cu{j}_{qc}")
                    nc.vector.tensor_copy(cu[0:64, :], ctxA[0:64, :])
                    cuB = cup.tile([P, 512], F32, tag="cuB", name=f"cB{j}_{qc}")
                    nc.vector.tensor_copy(cuB[0:64, :], ctxB[0:64, :])

                    def d1(j=j, qc=qc, rc=rc):
                        nc.vector.reciprocal_approx_fast(rc[:, 1024:2048],
                                                         rc[:, 0:1024])
                        bc = ps_s.tile([P, 1024], F32, tag="s",
                                       name=f"bc{j}_{qc}")
                        nc.tensor.matmul(bc[0:64, 0:512], ones_sb[64:65, :],
                                         rc[64:65, 1024:1536],
                                         start=True, stop=True)
                        nc.tensor.matmul(bc[0:64, 512:1024], ones_sb[64:65, :],
                                         rc[64:65, 1536:2048],
                                         start=True, stop=True)
                        st_norm[j, qc] = bc

                    def d2(j=j, qc=qc, qs=qs, cu=cu, cuB=cuB):
                        bc = st_norm.pop((j, qc))
                        nc.vector.tensor_mul(ctx_bf[j][0:64, qs], cu[0:64, :],
                                             bc[0:64, 0:512])
                        tmpB = cup.tile([P, 512], BF16, tag="tB",
                                        name=f"tB{j}_{qc}")
                        nc.vector.tensor_mul(tmpB[0:64, :], cuB[0:64, :],
                                             bc[0:64, 512:1024])
                        nc.sync.dma_start(ctx_bf[j][64:128, qs], tmpB[0:64, :])
                    fillers.extendleft([(150.0, d2, j), (760.0, d1, j)])
                    if j == 3 and parts == 'full':
                        fillers.extend(make_oproj_fillers(qc))

            # tail: drain remaining fillers (out-proj of the last q chunk)
            while fillers:
                fillers.popleft()[1]()


def build(loop_k: int = 1, parts: str = 'full'):
    nc = bacc.Bacc("TRN2", target_bir_lowering=False, debug=False)
    io = {
        "xT": nc.dram_tensor("xT", [D, T], BF16, kind="ExternalInput").ap(),
        "wq": nc.dram_tensor("wq", [D, G], BF16, kind="ExternalInput").ap(),
        "wk": nc.dram_tensor("wk", [D, G], BF16, kind="ExternalInput").ap(),
        "wv": nc.dram_tensor("wv", [D, VW], BF16, kind="ExternalInput").ap(),
        "wo": nc.dram_tensor("wo", [G, D], BF16, kind="ExternalInput").ap(),
        "bq": nc.dram_tensor("bq", [P, 4], F32, kind="ExternalInput").ap(),
        "bk": nc.dram_tensor("bk", [P, 4], F32, kind="ExternalInput").ap(),
        "bv": nc.dram_tensor("bv", [P, VW], F32, kind="ExternalInput").ap(),
        "out": nc.dram_tensor("out", [T, D], BF16, kind="ExternalOutput").ap(),
    }
    with tile.TileContext(nc) as tc:
        if loop_k == 1:
            emit_body(tc, io, parts)
        else:
            with tc.For_i(0, loop_k, 1):
                emit_body(tc, io, parts)
    nc.compile()
    return nc


def prep_inputs(x, Wq, bq, Wk, bk, Wv, bv, Wo, bo):
    """Host-side sharding: returns in_maps for cores 0..7."""
    f = np.float32
    bf = ml_dtypes.bfloat16
    in_maps = []
    for c in range(8):
        b, g = c // 2, c % 2
        gs = slice(g * G, (g + 1) * G)
        wv_aug = np.zeros((D, VW), f)
        bv_aug = np.zeros((VW,), f)
        wv_g = np.ascontiguousarray(Wv[gs, :].T)        # [D, 512]
        for h in range(GH):      # per head: [v(64), ones] -> S at row 64
            base = h * 65
            wv_aug[:, base:base + 64] = wv_g[:, h * 64:(h + 1) * 64]
            bv_aug[base:base + 64] = bv[gs][h * 64:(h + 1) * 64]
            bv_aug[base + 64] = 1.0
        in_maps.append({
            "xT": np.ascontiguousarray(np.asarray(x[b]).T).astype(bf),
            "wq": np.ascontiguousarray(Wq[gs, :].T).astype(bf),
            "wk": np.ascontiguousarray(Wk[gs, :].T).astype(bf),
            "wv": wv_aug.astype(bf),
            "wo": np.ascontiguousarray(Wo[:, gs].T).astype(bf),
            "bq": np.ascontiguousarray(bq[gs].reshape(4, P).T),
            "bk": np.ascontiguousarray(bk[gs].reshape(4, P).T),
            "bv": np.broadcast_to(bv_aug, (P, VW)).copy(),
        })
    return in_maps


def gather_output(results, bo):
    out = np.empty((B, T, D), np.float32)
    for b in range(B):
        out[b] = (results[2 * b]["out"].astype(np.float32)
                  + results[2 * b + 1]["out"].astype(np.float32)
                  + np.asarray(bo)[None, :])
    return out


_nc_cache = {}


def kernel(x, Wq, bq, Wk, bk, Wv, bv, Wo, bo):
    if "nc" not in _nc_cache:
        _nc_cache["nc"] = build()
    nc = _nc_cache["nc"]
    in_maps = prep_inputs(x, Wq, bq, Wk, bk, Wv, bv, Wo, bo)
    res = run_bass_kernel_spmd(nc, in_maps, list(range(8)))
    return gather_output(res.results, bo)
